# revision 14
# baseline (speedup 1.0000x reference)
"""Contrastive loss kernel for Trainium2, distributed over 8 NeuronCores.

Math: with MARGIN = 1.0 the reference loss
    mean(where(same, (1-c)^2, max(1-c, 0)^2))   with c = cos_sim(x_i, x_j)
collapses to mean((1-c)^2) for every pair, because c <= 1 always (so the
hinge max() is the identity) and both branches are then (1-c)^2 — labels
cannot change the value.  Expanding the square:
    N^2 * loss = N^2 - 2*sum_ij(c_ij) + sum_ij(c_ij^2)
               = N^2 - 2*||s||^2 + ||G||_F^2
with s = sum_r xn_r (a D-vector) and G = Xn^T Xn (D x D), Xn the
row-normalized reps.  This removes the N x N similarity matrix entirely.

Sharding: rows of reps are split across 8 cores (1024 rows each).  Each
core normalizes its rows and emits its partial Gram matrix G_c (upper
block-triangle only; G is symmetric) plus its partial column-sum s_c.
The host sums the partials and folds them into the scalar.

Hardware constraints that shaped the code:
  - TPB compute instructions accept at most 2 sync waits, HWDGE direct
    DMAs only 1.  Pools are sized so loop tiles are never reused (no WAR
    waits) and the total DMA count stays <= 8 (no queue-reuse waits).
  - The G blocks are written to DRAM packed as [128, 1280] (block a of
    shape [128, 512 - 128a] at column offset sum of prior widths), split
    over two staging tiles so each output DMA depends on one engine only.
"""

import numpy as np

import concourse.bass as bass
import concourse.tile as tile
from concourse import mybir
from concourse.bass_utils import run_bass_kernel_spmd

N = 8192
D = 512
N_CORES = 8
R = N // N_CORES          # rows per core
P = 128                   # partitions
N_CHUNKS = R // P         # row chunks per core
N_BLK = D // P            # 128-column blocks of G
EPS = 1e-8

# packed widths/offsets of the upper-triangle blocks of G
BLK_W = [D - a * P for a in range(N_BLK)]            # [512, 384, 256, 128]
# ga holds blocks 0 and 3 (ScalarE copies), gb blocks 1 and 2 (VectorE)
GA_BLOCKS = [0, 3]
GB_BLOCKS = [1, 2]
GA_W = sum(BLK_W[a] for a in GA_BLOCKS)              # 640
GB_W = sum(BLK_W[a] for a in GB_BLOCKS)              # 640

_NC = None


def build_nc() -> bass.Bass:
    nc = bass.Bass()
    f32 = mybir.dt.float32
    bf16 = mybir.dt.bfloat16

    x = nc.dram_tensor("x", [R, D], f32, kind="ExternalInput")
    ga_out = nc.dram_tensor("ga", [P, GA_W], f32, kind="ExternalOutput")
    gb_out = nc.dram_tensor("gb", [P, GB_W], f32, kind="ExternalOutput")
    s_out = nc.dram_tensor("s", [1, D], f32, kind="ExternalOutput")

    # two row-chunks per DMA: chunk i=2c+j lives at x2[c][:, j, :]
    x2 = x.rearrange("(c n p) d -> c p n d", n=2, p=P)

    with tile.TileContext(nc) as tc:
        with (
            tc.tile_pool(name="io", bufs=N_CHUNKS // 2) as io_pool,
            tc.tile_pool(name="xn", bufs=N_CHUNKS) as xn_pool,
            tc.tile_pool(name="scratch", bufs=1) as scratch_pool,
            tc.tile_pool(name="small", bufs=N_CHUNKS) as small_pool,
            tc.tile_pool(name="const", bufs=1) as const_pool,
            tc.tile_pool(name="gout", bufs=1) as gout_pool,
            tc.tile_pool(name="psum", bufs=1, space="PSUM") as psum_pool,
        ):
            ones_t = const_pool.tile([P, 1], bf16)
            nc.vector.memset(ones_t[:], 1.0)

            # Upper block-triangle of G: block row a covers columns a*128..D.
            psum_g = [
                psum_pool.tile([P, BLK_W[a]], f32, tag=f"psum_g{a}", name=f"psum_g{a}")
                for a in range(N_BLK)
            ]
            psum_s = psum_pool.tile([1, D], f32, tag="psum_s")

            for c in range(N_CHUNKS // 2):
                xt = io_pool.tile([P, 2, D], f32)
                nc.sync.dma_start(xt[:], x2[c])

                for j in range(2):
                    i = 2 * c + j
                    xv = xt[:, j, :]

                    # This toolchain's NEFF backend accepts at most ONE sync
                    # wait per TPB instruction, so the dependency graph is a
                    # strict relay: ScalarE owns every read of xt (square,
                    # scale-mul), DVE only touches the [P,1] norm tiles.
                    sq = scratch_pool.tile([P, D], f32, tag="sq")
                    ss = small_pool.tile([P, 1], f32, tag="ss")
                    nc.scalar.activation(
                        sq[:],
                        xv,
                        mybir.ActivationFunctionType.Square,
                        accum_out=ss[:],
                    )
                    nrm = small_pool.tile([P, 1], f32, tag="nrm")
                    nc.scalar.sqrt(nrm[:], ss[:])
                    nrmc = small_pool.tile([P, 1], f32, tag="nrmc")
                    nc.vector.tensor_scalar_max(nrmc[:], nrm[:], EPS)
                    rec = small_pool.tile([P, 1], f32, tag="rec")
                    nc.vector.reciprocal(rec[:], nrmc[:])

                    xn = xn_pool.tile([P, D], bf16)
                    nc.scalar.mul(xn[:], xv, rec[:])

                    first = i == 0
                    last = i == N_CHUNKS - 1
                    for a in range(N_BLK):
                        nc.tensor.matmul(
                            psum_g[a][:],
                            xn[:, a * P : (a + 1) * P],
                            xn[:, a * P : D],
                            start=first,
                            stop=last,
                        )
                    nc.tensor.matmul(
                        psum_s[:], ones_t[:], xn[:], start=first, stop=last
                    )

            # Drain PSUM into two packed staging tiles, one engine each, so
            # each output DMA waits on exactly one producer.
            ga = gout_pool.tile([P, GA_W], f32)
            off = 0
            for a in GA_BLOCKS:
                nc.scalar.copy(ga[:, off : off + BLK_W[a]], psum_g[a][:])
                off += BLK_W[a]
            gb = gout_pool.tile([P, GB_W], f32)
            off = 0
            for a in GB_BLOCKS:
                nc.vector.tensor_copy(gb[:, off : off + BLK_W[a]], psum_g[a][:])
                off += BLK_W[a]
            nc.sync.dma_start(ga_out[:], ga[:])
            nc.sync.dma_start(gb_out[:], gb[:])

            st = small_pool.tile([1, D], f32, tag="st")
            nc.vector.tensor_copy(st[:], psum_s[:])
            nc.sync.dma_start(s_out[:], st[:])

    _split_multi_waits(nc)
    mybir.codegen_inst_isa_subclasses(nc)
    return nc


def _split_multi_waits(nc: bass.Bass) -> None:
    """The NEFF backend here accepts at most one sync wait per TPB
    instruction, but Tile's kernel-tail drain carries one wait per live
    proc.  Split: hoist all but the last wait onto single-wait Drain
    instructions on the same engine, inserted immediately before."""
    n = 0
    for fn in nc.m.functions:
        for blk in fn.blocks:
            insts = blk.instructions
            i = 0
            while i < len(insts):
                inst = insts[i]
                si = inst.sync_info
                if si is not None and si.on_wait and len(si.on_wait) > 1:
                    waits = list(si.on_wait)
                    for w in waits[:-1]:
                        d = mybir.InstDrain(
                            name=f"I-waitsplit-{n}",
                            engine=inst.engine,
                            ins=[],
                            outs=[],
                            sync_info=mybir.SyncInfo(on_wait=[w], on_update=[]),
                        )
                        n += 1
                        nc.register_instruction(d)
                        insts.insert(i, d)
                        i += 1
                    inst.sync_info = mybir.SyncInfo(
                        on_wait=[waits[-1]], on_update=list(si.on_update or [])
                    )
                i += 1


def _get_nc() -> bass.Bass:
    global _NC
    if _NC is None:
        _NC = build_nc()
    return _NC


def _unpack_g(ga: np.ndarray, gb: np.ndarray) -> np.ndarray:
    """Scatter the packed staging buffers back into a [D, D] upper
    block-triangle (unwritten region left zero)."""
    g = np.zeros((D, D), dtype=np.float64)
    for src, blocks in ((ga, GA_BLOCKS), (gb, GB_BLOCKS)):
        off = 0
        for a in blocks:
            w = BLK_W[a]
            g[a * P : (a + 1) * P, a * P : D] += src[:, off : off + w]
            off += w
    return g


def kernel(reps: np.ndarray, labels: np.ndarray) -> np.ndarray:
    # Labels provably do not affect the loss (see module docstring).
    reps = np.ascontiguousarray(np.asarray(reps, dtype=np.float32))
    assert reps.shape == (N, D)

    in_maps = [
        {"x": np.ascontiguousarray(reps[c * R : (c + 1) * R])}
        for c in range(N_CORES)
    ]
    results = run_bass_kernel_spmd(_get_nc(), in_maps, list(range(N_CORES))).results

    g = np.zeros((D, D), dtype=np.float64)
    s = np.zeros((1, D), dtype=np.float64)
    for r in results:
        g += _unpack_g(r["ga"], r["gb"])
        s += r["s"]

    # g holds the upper block-triangle; the full Frobenius norm doubles the
    # off-diagonal blocks (symmetry) and counts diagonal blocks once.
    s2 = 0.0
    for a in range(N_BLK):
        row = g[a * P : (a + 1) * P, a * P : D]
        diag = g[a * P : (a + 1) * P, a * P : (a + 1) * P]
        s2 += 2.0 * float((row * row).sum()) - float((diag * diag).sum())
    s1 = float((s * s).sum())

    loss = (float(N) * N - 2.0 * s1 + s2) / (float(N) * N)
    return np.array(loss, dtype=np.float32)
